# revision 1
# baseline (speedup 1.0000x reference)
"""Trainium2 Bass kernel for a 2-layer mean-aggregation GCN + dot-product scoring.

Reference computation (per layer l in {0,1}):
    agg  = segment_sum(h[src], dst) / max(deg, 1)      # mean over incoming edges
    h    = tanh(agg @ Wl.T + bl)
Then:
    score[b, j] = sum_d h[user_index[b, j], d] * h[item_index[b, j], d]

Distribution strategy (8 NeuronCores):
  * Edges are sorted by dst on the host; each core owns a contiguous range of
    n_nodes/8 destination nodes and the edges that point into it.
  * The per-layer gather table g = h @ W.T (weight folded in: A(hW^T)=(Ah)W^T)
    is replicated in every core's DRAM via AllGather, fp32 with a 65th "ones"
    column so one matmul accumulates both feature sums and degree counts.
  * Per 128-dst tile, edges are fetched 128 at a time with indirect DMA
    (one row per partition), a one-hot [128e x 128dst] is built on DVE
    (iota == dstloc), and the segment sum accumulates in PSUM via PE matmuls:
    acc += onehot.T @ feats.
  * Mean + bias + tanh on ACT/DVE; layer-0 output is immediately multiplied by
    W1.T (PE) to form the next gather table; layer-1 output (fp32) is
    AllGathered and the final user/item rows are gathered + dotted on DVE.
"""

import numpy as np

P = 128

DEFAULT_CFG = dict(
    n_nodes=50000,
    d=64,
    n_edges=1250000,
    batch=1024,
    k=100,
    n_cores=8,
    kb=9,    # edge-tiles per one-hot DVE op
    kc=50,   # scoring chunk
)


def derived(cfg):
    nn_ = cfg["n_nodes"]
    ncores = cfg["n_cores"]
    assert nn_ % ncores == 0
    npc = nn_ // ncores              # dst nodes per core
    nt = -(-npc // P)                # dst tiles per core
    assert cfg["batch"] % ncores == 0
    rows = cfg["batch"] // ncores    # score rows per core
    assert rows <= P
    return npc, nt, rows


def preprocess(inputs, cfg):
    """Host-side sharding / index prep. Returns (in_maps, et)."""
    emb = np.asarray(inputs["embeddings"], np.float32)
    W0 = np.asarray(inputs["W0"], np.float32)
    b0 = np.asarray(inputs["b0"], np.float32)
    W1 = np.asarray(inputs["W1"], np.float32)
    b1 = np.asarray(inputs["b1"], np.float32)
    src = np.asarray(inputs["src"])
    dst = np.asarray(inputs["dst"])
    user_index = np.asarray(inputs["user_index"])
    item_index = np.asarray(inputs["item_index"])

    nn_, d = cfg["n_nodes"], cfg["d"]
    ncores = cfg["n_cores"]
    k = cfg["k"]
    npc, nt, rows = derived(cfg)

    order = np.argsort(dst, kind="stable")
    src_s = src[order].astype(np.int64)
    dst_s = dst[order].astype(np.int64)

    tile_lo_node, tile_hi_node = [], []
    for c in range(ncores):
        for t in range(nt):
            lo = c * npc + t * P
            hi = min(c * npc + npc, lo + P)
            tile_lo_node.append(lo)
            tile_hi_node.append(hi)
    los = np.searchsorted(dst_s, np.array(tile_lo_node))
    his = np.searchsorted(dst_s, np.array(tile_hi_node))
    cnts = his - los
    tiles_needed = (cnts + P - 1) // P               # per (core, t)
    et = int(max(1, tiles_needed.max()))             # uniform layout stride
    # per-t tile count = max over cores (same instruction stream on every core,
    # but position t only runs as many edge-tiles as its worst core needs)
    et_list = tuple(
        int(max(1, tiles_needed.reshape(ncores, nt)[:, t].max())) for t in range(nt)
    )

    # padding edge slots gather row 0 (their one-hot row is all-zero)
    src_idx = np.zeros((ncores, P, nt * et), np.int32)
    dstloc = np.full((ncores, P, nt * et), 300.0, np.float32)
    for g in range(ncores * nt):
        c, t = divmod(g, nt)
        lo, hi = los[g], his[g]
        n = hi - lo
        if n == 0:
            continue
        e = np.arange(n)
        j = e // P
        p = e % P
        src_idx[c, p, t * et + j] = src_s[lo:hi]
        dstloc[c, p, t * et + j] = dst_s[lo:hi] - (c * npc + t * P)

    W0t = np.ascontiguousarray(W0.T)
    W1t = np.ascontiguousarray(W1.T)
    b0r = np.ascontiguousarray(np.broadcast_to(b0, (P, d))).astype(np.float32)
    b1r = np.ascontiguousarray(np.broadcast_to(b1, (P, d))).astype(np.float32)

    in_maps = []
    for c in range(ncores):
        embT = np.zeros((d, nt * P), np.float32)
        embT[:, :npc] = emb[c * npc:(c + 1) * npc].T
        # scoring: flatten this core's (row, k) pairs; column j holds pairs
        # [j*P, (j+1)*P) so each gather is one index per partition
        ui = user_index[c * rows:(c + 1) * rows].astype(np.int32).ravel()
        ii = item_index[c * rows:(c + 1) * rows].astype(np.int32).ravel()
        ui = np.ascontiguousarray(ui.reshape(rows * k // P, P).T)
        ii = np.ascontiguousarray(ii.reshape(rows * k // P, P).T)
        in_maps.append(dict(
            embT=embT,
            W0t=W0t, W1t=W1t, b0r=b0r, b1r=b1r,
            src_idx=src_idx[c], dstloc=dstloc[c],
            user_idx=ui, item_idx=ii,
        ))
    return in_maps, et, et_list


def build_nc(cfg, et, et_list=None):
    """Builds + compiles the Bass program. Returns nc."""
    import concourse.bass as bass
    import concourse.bacc as bacc
    import concourse.mybir as mybir
    import concourse.tile as tile
    from concourse.masks import make_identity

    f32 = mybir.dt.float32
    i32 = mybir.dt.int32

    nn_, d = cfg["n_nodes"], cfg["d"]
    dv = d + 1
    ncores = cfg["n_cores"]
    k = cfg["k"]
    npc, nt, rows = derived(cfg)
    groups = [list(range(ncores))]
    nsc = rows * k // P              # scoring gather columns (pairs / P)
    if et_list is None:
        et_list = tuple([et] * nt)

    nc = bacc.Bacc(
        "TRN2",
        target_bir_lowering=False,
        debug=False,
        enable_asserts=False,
        num_devices=ncores,
    )

    # ---------------- I/O ----------------
    embT_d = nc.dram_tensor("embT", [d, nt * P], f32, kind="ExternalInput")
    W0t_d = nc.dram_tensor("W0t", [d, d], f32, kind="ExternalInput")
    W1t_d = nc.dram_tensor("W1t", [d, d], f32, kind="ExternalInput")
    b0r_d = nc.dram_tensor("b0r", [P, d], f32, kind="ExternalInput")
    b1r_d = nc.dram_tensor("b1r", [P, d], f32, kind="ExternalInput")
    sidx_d = nc.dram_tensor("src_idx", [P, nt * et], i32, kind="ExternalInput")
    dloc_d = nc.dram_tensor("dstloc", [P, nt * et], f32, kind="ExternalInput")
    uidx_d = nc.dram_tensor("user_idx", [P, nsc], i32, kind="ExternalInput")
    iidx_d = nc.dram_tensor("item_idx", [P, nsc], i32, kind="ExternalInput")
    score_d = nc.dram_tensor("score", [P, nsc], f32, kind="ExternalOutput")

    g0_loc = nc.dram_tensor("g0_loc", [npc, dv], f32)
    g1_loc = nc.dram_tensor("g1_loc", [npc, dv], f32)
    h1_loc = nc.dram_tensor("h1_loc", [npc, d], f32)
    g0_full = nc.dram_tensor("g0_full", [nn_, dv], f32, addr_space="Shared")
    g1_full = nc.dram_tensor("g1_full", [nn_, dv], f32, addr_space="Shared")
    h1_full = nc.dram_tensor("h1_full", [nn_, d], f32, addr_space="Shared")

    with tile.TileContext(nc) as tc:
        with (
            tc.tile_pool(name="const", bufs=1) as cpool,
            tc.tile_pool(name="gath", bufs=96) as gpool,
            tc.tile_pool(name="oh", bufs=2) as ohpool,
            tc.tile_pool(name="work", bufs=3) as wpool,
            tc.tile_pool(name="stage", bufs=3) as spool,
            tc.tile_pool(name="sco", bufs=1) as scpool,
            tc.tile_pool(name="pacc", bufs=3, space="PSUM") as pacc,
            tc.tile_pool(name="ptp", bufs=2, space="PSUM") as ptp,
            tc.tile_pool(name="pg", bufs=2, space="PSUM") as pg,
        ):
            # ---------- constants ----------
            ident = cpool.tile([P, P], f32)
            make_identity(nc, ident[:])
            iota_i = cpool.tile([P, P], i32)
            nc.gpsimd.iota(iota_i[:], pattern=[[1, P]], base=0, channel_multiplier=0)
            iota_f = cpool.tile([P, P], f32)
            nc.vector.tensor_copy(iota_f[:], iota_i[:])

            embT_sb = cpool.tile([d, nt * P], f32)
            nc.sync.dma_start(embT_sb[:], embT_d[:, :])
            W0t_sb = cpool.tile([d, d], f32)
            nc.sync.dma_start(W0t_sb[:], W0t_d[:, :])
            W1t_sb = cpool.tile([d, d], f32)
            nc.sync.dma_start(W1t_sb[:], W1t_d[:, :])
            b0_sb = cpool.tile([P, d], f32)
            nc.sync.dma_start(b0_sb[:], b0r_d[:, :])
            b1_sb = cpool.tile([P, d], f32)
            nc.sync.dma_start(b1_sb[:], b1r_d[:, :])
            sidx_sb = cpool.tile([P, nt * et], i32)
            nc.sync.dma_start(sidx_sb[:], sidx_d[:, :])
            dloc_sb = cpool.tile([P, nt * et], f32)
            nc.sync.dma_start(dloc_sb[:], dloc_d[:, :])
            uidx_sb = cpool.tile([P, nsc], i32)
            nc.sync.dma_start(uidx_sb[:], uidx_d[:, :])
            iidx_sb = cpool.tile([P, nsc], i32)
            nc.sync.dma_start(iidx_sb[:], iidx_d[:, :])

            def stage_g(gsrc_psum, dest_dram, t):
                gst = spool.tile([P, dv], f32, tag="gstage")
                nc.vector.tensor_copy(gst[:, :d], gsrc_psum[:])
                nc.vector.memset(gst[:, d:dv], 1.0)
                r = min(P, npc - t * P)
                nc.sync.dma_start(dest_dram[t * P:t * P + r, :], gst[:r, :])

            # ---------- g0 = emb @ W0.T ----------
            for t in range(nt):
                g0p = pg.tile([P, d], f32, tag="pg")
                nc.tensor.matmul(
                    g0p[:], lhsT=embT_sb[:, t * P:(t + 1) * P], rhs=W0t_sb[:],
                    start=True, stop=True,
                )
                stage_g(g0p, g0_loc, t)

            nc.gpsimd.collective_compute(
                "AllGather", mybir.AluOpType.bypass, replica_groups=groups,
                ins=[g0_loc[:, :]], outs=[g0_full[:, :]],
            )

            # ---------- layers ----------
            kb = cfg["kb"]

            def layer(gtab, b_sb, is_last):
                for t in range(nt):
                    ett = et_list[t]
                    gaths = []
                    for j in range(ett):
                        gt = gpool.tile([P, dv], f32, tag="gath")
                        nc.gpsimd.indirect_dma_start(
                            out=gt[:, :],
                            out_offset=None,
                            in_=gtab[:, :],
                            in_offset=bass.IndirectOffsetOnAxis(
                                ap=sidx_sb[:, t * et + j:t * et + j + 1], axis=0,
                            ),
                        )
                        gaths.append(gt)
                    oh = ohpool.tile([P, et, P], f32, tag="oh")
                    for j0 in range(0, ett, kb):
                        jn = min(kb, ett - j0)
                        nc.vector.tensor_tensor(
                            out=oh[:, j0:j0 + jn, :],
                            in0=iota_f[:, None, :].broadcast_to([P, jn, P]),
                            in1=dloc_sb[:, t * et + j0:t * et + j0 + jn][:, :, None]
                                .broadcast_to([P, jn, P]),
                            op=mybir.AluOpType.is_equal,
                        )
                    acc = pacc.tile([P, dv], f32, tag="pacc")
                    for j in range(ett):
                        nc.tensor.matmul(
                            acc[:], lhsT=oh[:, j, :], rhs=gaths[j][:, :],
                            start=(j == 0), stop=(j == ett - 1),
                        )
                    # mean, bias, tanh
                    degc = wpool.tile([P, 1], f32, tag="degc")
                    nc.vector.tensor_scalar_max(degc[:], acc[:, d:dv], 1.0)
                    recip = wpool.tile([P, 1], f32, tag="recip")
                    nc.vector.reciprocal(recip[:], degc[:])
                    mean = wpool.tile([P, d], f32, tag="mean")
                    nc.scalar.activation(
                        mean[:], acc[:, :d],
                        mybir.ActivationFunctionType.Copy, scale=recip[:],
                    )
                    z = wpool.tile([P, d], f32, tag="z")
                    nc.vector.tensor_add(z[:], mean[:], b_sb[:])
                    h = wpool.tile([P, d], f32, tag="h")
                    nc.scalar.activation(h[:], z[:], mybir.ActivationFunctionType.Tanh)
                    r = min(P, npc - t * P)
                    if is_last:
                        nc.sync.dma_start(h1_loc[t * P:t * P + r, :], h[:r, :])
                    else:
                        tp = ptp.tile([d, P], f32, tag="ptp")
                        nc.tensor.transpose(tp[:], h[:], ident[:])
                        hT = wpool.tile([d, P], f32, tag="hT")
                        nc.vector.tensor_copy(hT[:], tp[:])
                        gm = pg.tile([P, d], f32, tag="pg")
                        nc.tensor.matmul(gm[:], lhsT=hT[:], rhs=W1t_sb[:],
                                         start=True, stop=True)
                        stage_g(gm, g1_loc, t)

            layer(g0_full, b0_sb, is_last=False)
            nc.gpsimd.collective_compute(
                "AllGather", mybir.AluOpType.bypass, replica_groups=groups,
                ins=[g1_loc[:, :]], outs=[g1_full[:, :]],
            )
            layer(g1_full, b1_sb, is_last=True)
            nc.gpsimd.collective_compute(
                "AllGather", mybir.AluOpType.bypass, replica_groups=groups,
                ins=[h1_loc[:, :]], outs=[h1_full[:, :]],
            )

            # ---------- scoring ----------
            kcn = cfg["kc"] * rows // P          # gather columns per chunk
            sc_out = cpool.tile([P, nsc], f32)
            for c0 in range(0, nsc, kcn):
                cn = min(kcn, nsc - c0)
                ug = scpool.tile([P, kcn, d], f32, tag="ug")
                ig = scpool.tile([P, kcn, d], f32, tag="ig")
                for j in range(cn):
                    nc.gpsimd.indirect_dma_start(
                        out=ug[:, j, :], out_offset=None, in_=h1_full[:, :],
                        in_offset=bass.IndirectOffsetOnAxis(
                            ap=uidx_sb[:, c0 + j:c0 + j + 1], axis=0),
                    )
                    nc.gpsimd.indirect_dma_start(
                        out=ig[:, j, :], out_offset=None, in_=h1_full[:, :],
                        in_offset=bass.IndirectOffsetOnAxis(
                            ap=iidx_sb[:, c0 + j:c0 + j + 1], axis=0),
                    )
                prod = scpool.tile([P, kcn, d], f32, tag="prod")
                nc.vector.tensor_tensor(
                    out=prod[:, :cn, :], in0=ug[:, :cn, :], in1=ig[:, :cn, :],
                    op=mybir.AluOpType.mult,
                )
                nc.vector.tensor_reduce(
                    out=sc_out[:, c0:c0 + cn], in_=prod[:, :cn, :],
                    axis=mybir.AxisListType.X, op=mybir.AluOpType.add,
                )
            nc.sync.dma_start(score_d[:, :], sc_out[:])

    nc.compile()
    return nc


class _SpmdRunner:
    """Mirrors bass2jax.run_bass_via_pjrt but caches the jitted executable so
    repeated kernel() calls skip re-tracing."""

    def __init__(self, nc, n_cores):
        import jax
        from jax.sharding import Mesh, PartitionSpec
        from jax.experimental.shard_map import shard_map
        import concourse.mybir as mybir
        from concourse import bass2jax
        from concourse.bass2jax import _bass_exec_p, install_neuronx_cc_hook

        install_neuronx_cc_hook()
        self.jax = jax
        self.n_cores = n_cores
        partition_name = (
            nc.partition_id_tensor.name if nc.partition_id_tensor else None
        )
        in_names, out_names, out_avals, zero_outs = [], [], [], []
        for alloc in nc.m.functions[0].allocations:
            if not isinstance(alloc, mybir.MemoryLocationSet):
                continue
            name = alloc.memorylocations[0].name
            if alloc.kind == "ExternalInput":
                if name != partition_name:
                    in_names.append(name)
            elif alloc.kind == "ExternalOutput":
                out_names.append(name)
                shape = tuple(alloc.tensor_shape)
                dtype = mybir.dt.np(alloc.dtype)
                out_avals.append(jax.core.ShapedArray(shape, dtype))
                zero_outs.append(np.zeros(shape, dtype))
        self.in_names, self.out_names = in_names, out_names
        self.out_avals, self.zero_outs = out_avals, zero_outs
        all_in_names = list(in_names) + list(out_names)
        if partition_name is not None:
            all_in_names.append(partition_name)

        def _body(*args):
            operands = list(args)
            if partition_name is not None:
                operands.append(bass2jax.partition_id_tensor())
            return tuple(_bass_exec_p.bind(
                *operands,
                out_avals=tuple(out_avals),
                in_names=tuple(all_in_names),
                out_names=tuple(out_names),
                lowering_input_output_aliases=(),
                sim_require_finite=True,
                sim_require_nnan=True,
                nc=nc,
            ))

        devices = jax.devices()[:n_cores]
        self.mesh = Mesh(np.asarray(devices), ("core",))
        n_io = len(in_names) + len(out_names)
        self.fn = jax.jit(
            shard_map(_body, mesh=self.mesh,
                      in_specs=(PartitionSpec("core"),) * n_io,
                      out_specs=(PartitionSpec("core"),) * len(out_names),
                      check_rep=False),
            keep_unused=True,
        )

    def run(self, in_maps):
        concat_in = [
            np.concatenate([np.asarray(in_maps[c][nm]) for c in range(self.n_cores)],
                           axis=0)
            for nm in self.in_names
        ]
        concat_zeros = [
            np.zeros((self.n_cores * z.shape[0], *z.shape[1:]), z.dtype)
            for z in self.zero_outs
        ]
        outs = self.fn(*concat_in, *concat_zeros)
        self.jax.block_until_ready(outs)
        res = [dict() for _ in range(self.n_cores)]
        for i, nm in enumerate(self.out_names):
            full = np.asarray(outs[i]).reshape(self.n_cores, *self.out_avals[i].shape)
            for c in range(self.n_cores):
                res[c][nm] = full[c]
        return res


_CACHE = {}


def _get_runner(cfg_key, cfg, et, et_list):
    key = (cfg_key, et, et_list)
    if key not in _CACHE:
        nc = build_nc(cfg, et, et_list)
        _CACHE[key] = _SpmdRunner(nc, cfg["n_cores"])
    return _CACHE[key]


def assemble_score(results, cfg):
    npc, nt, rows = derived(cfg)
    k = cfg["k"]
    parts = []
    for r in results:
        sc = r["score"]                    # [P, nsc]; column j = pairs [j*P,(j+1)*P)
        parts.append(np.ascontiguousarray(sc.T).reshape(rows, k))
    return np.concatenate(parts, axis=0).astype(np.float32)


def run(inputs, cfg=None):
    """Returns (score [batch, k] float32, per-core results)."""
    cfg = dict(DEFAULT_CFG, **(cfg or {}))
    in_maps, et, et_list = preprocess(inputs, cfg)
    cfg_key = tuple(sorted((kk, v) for kk, v in cfg.items()))
    runner = _get_runner(cfg_key, cfg, et, et_list)
    results = runner.run(in_maps)
    return assemble_score(results, cfg), results


def kernel(**inputs) -> np.ndarray:
    score, _ = run(inputs)
    return score



# revision 31
# speedup vs baseline: 1.9142x; 1.9142x over previous
"""Trainium2 Bass kernel for a 2-layer mean-aggregation GCN + dot-product scoring.

Reference computation (per layer l in {0,1}):
    agg  = segment_sum(h[src], dst) / max(deg, 1)      # mean over incoming edges
    h    = tanh(agg @ Wl.T + bl)
Then:
    score[b, j] = sum_d h[user_index[b, j], d] * h[item_index[b, j], d]

Distribution / algorithm (8 NeuronCores):
  * Edges sorted by dst; core c owns dst nodes [c*npc, (c+1)*npc).
  * Gather tables hold g = h @ W.T rows (weight folded in before the gather)
    in bf16.  The layer-1 table g0 is computed locally by EVERY core from the
    replicated embedding input (no collective); g1 and the final h2 are each
    AllGathered once (bf16).
  * Edge features are fetched with the GPSIMD dma_gather instruction: the
    [n, 64] bf16 table is viewed as [n/2, 128] pairs so one 256B descriptor
    per edge fits dma_gather's int16 / 256B-stride constraints; each edge's
    int16 index is src>>1 and the parity src&1 selects which half of the
    fetched pair feeds the PE.  Indices live in SBUF wrapped over 16
    partitions and replicated for the 8 GPSIMD cores.
  * Segment sum per 128-dst tile: a one-hot [edge, dst] matrix built on DVE
    (iota == dstloc, bf16) is matmul-accumulated against the gathered rows
    in PSUM; edges in each tile are host-sorted by parity so each one-hot
    column uses a single rhs half.
  * mean/bias via one fused scalar_tensor_tensor (x*inv_deg + b) on DVE,
    tanh + PSUM->SBUF copies on ACT, transposes + W-fold matmuls on PE.
  * Scoring gathers user/item pairs with dma_gather as well; score pairs are
    host-sorted into 4 (user-parity, item-parity) groups so the elementwise
    product can slice fixed pair halves; the host un-permutes the output.
"""

import numpy as np

P = 128

DEFAULT_CFG = dict(
    n_nodes=50000,
    d=64,
    n_edges=1250000,
    batch=1024,
    k=100,
    n_cores=8,
    gwc=8,       # columns (of 128 edges) per dma_gather; 8*128=1024 indices
                 # stays within the SWDGE descriptor ring (larger crashes)
    g0b=8,       # g0 tiles per PSUM group
    g0f=32,      # g0 tiles per embT load / staging flush
    stb=4,       # layer staging tiles per flush
)


def derived(cfg):
    nn_ = cfg["n_nodes"]
    ncores = cfg["n_cores"]
    assert nn_ % ncores == 0
    npc = nn_ // ncores              # dst nodes per core
    nt = -(-npc // P)                # dst tiles per core
    assert cfg["batch"] % ncores == 0
    rows = cfg["batch"] // ncores    # score rows per core
    assert rows <= P
    return npc, nt, rows


def _wrap16(vals):
    """[n] int16 -> [128, n/16] wrapped over 16 partitions, replicated x8."""
    n = vals.shape[0]
    assert n % 16 == 0
    blk = np.zeros((16, n // 16), np.int16)
    blk[np.arange(n) % 16, np.arange(n) // 16] = vals
    return np.tile(blk, (8, 1))


def preprocess(inputs, cfg):
    """Host-side sharding / index prep. Returns (in_maps, meta)."""
    import concourse.mybir as mybir
    bf16_np = mybir.dt.np(mybir.dt.bfloat16)

    emb = np.asarray(inputs["embeddings"], np.float32)
    W0 = np.asarray(inputs["W0"], np.float32)
    b0 = np.asarray(inputs["b0"], np.float32)
    W1 = np.asarray(inputs["W1"], np.float32)
    b1 = np.asarray(inputs["b1"], np.float32)
    src = np.asarray(inputs["src"]).astype(np.int64)
    dst = np.asarray(inputs["dst"]).astype(np.int64)
    user_index = np.asarray(inputs["user_index"])
    item_index = np.asarray(inputs["item_index"])

    nn_, d = cfg["n_nodes"], cfg["d"]
    ncores = cfg["n_cores"]
    k = cfg["k"]
    npc, nt, rows = derived(cfg)

    deg = np.bincount(dst, minlength=nn_).astype(np.int64)
    invd_full = (1.0 / np.maximum(deg, 1.0)).astype(np.float32)

    order = np.argsort(dst, kind="stable")
    src_s = src[order]
    dst_s = dst[order]

    # ---- per-(core, tile, parity) edge groups ------------------------------
    # boundaries of each (core, tile) segment in the dst-sorted edge list
    tile_edges = {}
    ncol_e = np.zeros(nt, np.int64)
    ncol_o = np.zeros(nt, np.int64)
    for c in range(ncores):
        lo_all = np.searchsorted(dst_s, c * npc + np.arange(nt) * P)
        hi_all = np.searchsorted(
            dst_s, np.minimum(c * npc + (np.arange(nt) + 1) * P, (c + 1) * npc))
        for t in range(nt):
            lo, hi = lo_all[t], hi_all[t]
            s = src_s[lo:hi]
            dl = dst_s[lo:hi] - (c * npc + t * P)
            pe = s % 2 == 0
            tile_edges[(c, t)] = ((s[pe] >> 1, dl[pe]), (s[~pe] >> 1, dl[~pe]))
            ncol_e[t] = max(ncol_e[t], -(-int(pe.sum()) // P))
            ncol_o[t] = max(ncol_o[t], -(-int((~pe).sum()) // P))
    for t in range(nt):
        if ncol_e[t] + ncol_o[t] == 0:
            ncol_e[t] = 1
    tcols = [int(ncol_e[t] + ncol_o[t]) for t in range(nt)]
    col_base = np.concatenate([[0], np.cumsum(tcols)])
    CC = int(col_base[-1])
    gwc = cfg["gwc"]
    CCp = -(-CC // gwc) * gwc        # pad to whole gather windows

    pairidx = np.zeros((ncores, P, CCp), np.int16)
    dstloc = np.full((ncores, P, CCp), 300.0, np.float32)
    for c in range(ncores):
        for t in range(nt):
            (se, de), (so, do_) = tile_edges[(c, t)]
            cb = int(col_base[t])
            for vals, dl, cb0 in ((se, de, cb), (so, do_, cb + int(ncol_e[t]))):
                n = len(vals)
                if n == 0:
                    continue
                e = np.arange(n)
                pairidx[c, e % P, cb0 + e // P] = vals.astype(np.int16)
                dstloc[c, e % P, cb0 + e // P] = dl.astype(np.float32)

    # wrapped int16 index stream, one block per gather window
    sidx16 = np.zeros((ncores, P, CCp * 8), np.int16)
    for c in range(ncores):
        for w in range(CCp // gwc):
            c0 = w * gwc
            flat = np.ascontiguousarray(pairidx[c, :, c0:c0 + gwc].T).ravel()
            # position i = col*128 + p  ->  flat of [gwc, P]
            sidx16[c][:, c0 * 8:(c0 + gwc) * 8] = _wrap16(flat.astype(np.int16))

    # per-tile parity of each local column (ph list per tile)
    tile_par = [[0] * int(ncol_e[t]) + [1] * int(ncol_o[t]) for t in range(nt)]

    # inv-degree per (core, slot, tile)
    invd = np.ones((ncores, P, nt), np.float32)
    for c in range(ncores):
        blk = invd_full[c * npc:(c + 1) * npc]
        pad = np.ones(nt * P - npc, np.float32)
        invd[c] = np.concatenate([blk, pad]).reshape(nt, P).T

    # ---- scoring: 4 parity groups ------------------------------------------
    ugrp_cols = np.zeros(4, np.int64)
    score_data = []
    for c in range(ncores):
        u = user_index[c * rows:(c + 1) * rows].astype(np.int64).ravel()
        it = item_index[c * rows:(c + 1) * rows].astype(np.int64).ravel()
        g = (u % 2) * 2 + (it % 2)
        o = np.argsort(g, kind="stable")
        score_data.append((u[o], it[o], o, g[o]))
        for gi in range(4):
            ugrp_cols[gi] = max(ugrp_cols[gi], -(-int((g == gi).sum()) // P))
    sg_base = np.concatenate([[0], np.cumsum(ugrp_cols)])
    nsc2 = int(sg_base[-1])
    nsc2p = -(-nsc2 // gwc) * gwc
    sgroups = [(int(sg_base[gi]), int(ugrp_cols[gi]), gi // 2, gi % 2)
               for gi in range(4)]

    uidx16 = np.zeros((ncores, P, nsc2p * 8), np.int16)
    iidx16 = np.zeros((ncores, P, nsc2p * 8), np.int16)
    perms = []
    for c in range(ncores):
        u, it, o, g = score_data[c]
        perm = np.full(nsc2 * P, -1, np.int64)
        uflat = np.zeros(nsc2p * P, np.int64)
        iflat = np.zeros(nsc2p * P, np.int64)
        for gi in range(4):
            m = g == gi
            n = int(m.sum())
            s0 = int(sg_base[gi]) * P
            uflat[s0:s0 + n] = u[m] >> 1
            iflat[s0:s0 + n] = it[m] >> 1
            # pad rows gather pair 0 (valid data, discarded by perm)
            perm[s0:s0 + n] = o[m]
        for w in range(nsc2p // gwc):
            c0 = w * gwc
            ni = gwc * P
            # window block is column-major within the window
            ub = uflat[c0 * P:c0 * P + ni].astype(np.int16)
            ib = iflat[c0 * P:c0 * P + ni].astype(np.int16)
            uidx16[c][:, c0 * 8:(c0 + gwc) * 8] = _wrap16(ub)
            iidx16[c][:, c0 * 8:(c0 + gwc) * 8] = _wrap16(ib)
        perms.append(perm)

    # ---- weights / tables --------------------------------------------------
    embT = np.ascontiguousarray(emb.T).astype(bf16_np)
    W0t = np.ascontiguousarray(W0.T).astype(bf16_np)
    W1t = np.ascontiguousarray(W1.T).astype(bf16_np)
    b0r = np.ascontiguousarray(np.broadcast_to(b0, (P, d))).astype(np.float32)
    b1r = np.ascontiguousarray(np.broadcast_to(b1, (P, d))).astype(np.float32)

    in_maps = []
    for c in range(ncores):
        in_maps.append(dict(
            embT=embT, W0t=W0t, W1t=W1t, b0r=b0r, b1r=b1r,
            sidx16=sidx16[c], dloc=dstloc[c].astype(bf16_np), invd=invd[c],
            uidx16=uidx16[c], iidx16=iidx16[c],
        ))
    meta = dict(
        CC=CC, CCp=CCp, tcols=tuple(tcols),
        col_base=tuple(int(x) for x in col_base),
        ncol_e=tuple(int(x) for x in ncol_e),
        nsc2=nsc2, nsc2p=nsc2p, sgroups=tuple(sgroups),
        perms=perms,
    )
    return in_maps, meta


def build_nc(cfg, meta):
    """Builds + compiles the Bass program. Returns nc."""
    import concourse.bass as bass
    import concourse.bacc as bacc
    import concourse.mybir as mybir
    import concourse.tile as tile

    f32 = mybir.dt.float32
    bf16 = mybir.dt.bfloat16
    i16 = mybir.dt.int16
    i32 = mybir.dt.int32

    nn_, d = cfg["n_nodes"], cfg["d"]
    ncores = cfg["n_cores"]
    k = cfg["k"]
    npc, nt, rows = derived(cfg)
    groups = [list(range(ncores))]
    CC = meta["CC"]
    CCp = meta["CCp"]
    tcols = meta["tcols"]
    col_base = meta["col_base"]
    ncol_e = meta["ncol_e"]
    nsc2 = meta["nsc2"]
    nsc2p = meta["nsc2p"]
    sgroups = meta["sgroups"]
    gwc = cfg["gwc"]
    # per-column tile / parity / chain-position lookup
    tile_of_col = [None] * CCp
    par_of_col = [0] * CCp
    for t in range(nt):
        for j in range(tcols[t]):
            tile_of_col[col_base[t] + j] = t
            par_of_col[col_base[t] + j] = 0 if j < ncol_e[t] else 1
    ng0 = -(-nn_ // P)
    g0b = cfg["g0b"]
    g0f = cfg["g0f"]
    stb = cfg["stb"]

    nc = bacc.Bacc(
        "TRN2",
        target_bir_lowering=False,
        debug=False,
        enable_asserts=False,
        num_devices=ncores,
    )

    # ---------------- I/O ----------------
    embT_d = nc.dram_tensor("embT", [d, nn_], bf16, kind="ExternalInput")
    W0t_d = nc.dram_tensor("W0t", [d, d], bf16, kind="ExternalInput")
    W1t_d = nc.dram_tensor("W1t", [d, d], bf16, kind="ExternalInput")
    b0r_d = nc.dram_tensor("b0r", [P, d], f32, kind="ExternalInput")
    b1r_d = nc.dram_tensor("b1r", [P, d], f32, kind="ExternalInput")
    sidx_d = nc.dram_tensor("sidx16", [P, CCp * 8], i16, kind="ExternalInput")
    dloc_d = nc.dram_tensor("dloc", [P, CCp], bf16, kind="ExternalInput")
    invd_d = nc.dram_tensor("invd", [P, nt], f32, kind="ExternalInput")
    uidx_d = nc.dram_tensor("uidx16", [P, nsc2p * 8], i16, kind="ExternalInput")
    iidx_d = nc.dram_tensor("iidx16", [P, nsc2p * 8], i16, kind="ExternalInput")
    score_d = nc.dram_tensor("score", [P, nsc2], f32, kind="ExternalOutput")

    g0_full = nc.dram_tensor("g0_full", [nn_, d], bf16)
    g1_full = nc.dram_tensor("g1_full", [nn_, d], bf16, addr_space="Shared")
    h2_full = nc.dram_tensor("h2_full", [nn_, d], bf16, addr_space="Shared")
    g1_loc = nc.dram_tensor("g1_loc", [npc, d], bf16)
    h2_loc = nc.dram_tensor("h2_loc", [npc, d], bf16)

    with tile.TileContext(nc) as tc:
        with (
            tc.tile_pool(name="const", bufs=1) as cpool,
            tc.tile_pool(name="emb", bufs=2) as epool,
            tc.tile_pool(name="gath", bufs=3) as gpool,
            tc.tile_pool(name="oh", bufs=2) as ohpool,
            tc.tile_pool(name="work", bufs=4) as wpool,
            tc.tile_pool(name="stage", bufs=3) as spool,
            tc.tile_pool(name="sco", bufs=1) as scpool,
            tc.tile_pool(name="pacc", bufs=3, space="PSUM") as pacc,
            tc.tile_pool(name="pg", bufs=2, space="PSUM") as pg,
            tc.tile_pool(name="pg0", bufs=2, space="PSUM") as pg0pool,
            tc.tile_pool(name="ptp", bufs=1, space="PSUM") as ptp,
        ):
            # ---------- constants ----------
            from concourse.masks import make_identity
            ident = cpool.tile([P, P], f32)
            make_identity(nc, ident[:])
            iota_i = cpool.tile([P, P], i32)
            nc.gpsimd.iota(iota_i[:], pattern=[[1, P]], base=0,
                           channel_multiplier=0)
            iota_b = cpool.tile([P, P], bf16)
            nc.vector.tensor_copy(iota_b[:], iota_i[:])
            W0t_sb = cpool.tile([d, d], bf16)
            nc.sync.dma_start(W0t_sb[:], W0t_d[:, :])
            W1t_sb = cpool.tile([d, d], bf16)
            nc.sync.dma_start(W1t_sb[:], W1t_d[:, :])
            b0_sb = cpool.tile([P, d], f32)
            nc.sync.dma_start(b0_sb[:], b0r_d[:, :])
            b1_sb = cpool.tile([P, d], f32)
            nc.sync.dma_start(b1_sb[:], b1r_d[:, :])
            sidx_sb = cpool.tile([P, CCp * 8], i16)
            nc.sync.dma_start(sidx_sb[:], sidx_d[:, :])
            dloc_sb = cpool.tile([P, CCp], bf16)
            nc.sync.dma_start(dloc_sb[:], dloc_d[:, :])
            invd_sb = cpool.tile([P, nt], f32)
            nc.sync.dma_start(invd_sb[:], invd_d[:, :])
            uidx_sb = cpool.tile([P, nsc2p * 8], i16)
            nc.sync.dma_start(uidx_sb[:], uidx_d[:, :])
            iidx_sb = cpool.tile([P, nsc2p * 8], i16)
            nc.sync.dma_start(iidx_sb[:], iidx_d[:, :])

            # ---------- g0 = emb @ W0.T, computed fully on every core -------
            for gi0 in range(0, ng0, g0f):
                gn = min(g0f, ng0 - gi0)
                r0 = gi0 * P
                rn = min(nn_ - r0, gn * P)
                ech = epool.tile([d, g0f * P], bf16, tag="ech")
                nc.sync.dma_start(ech[:, :rn], embT_d[:, r0:r0 + rn])
                gst = spool.tile([P, g0f, d], bf16, tag="g0st")
                for j0 in range(0, gn, g0b):
                    jn = min(g0b, gn - j0)
                    g0p = pg0pool.tile([P, g0b, d], f32, tag="pg0")
                    jfull = jn
                    for j in range(j0, j0 + jn):
                        rj = min(P, nn_ - (r0 + j * P))
                        if rj < P:
                            jfull = j - j0
                        nc.tensor.matmul(
                            g0p[:rj, j - j0, :], lhsT=ech[:, j * P:j * P + rj],
                            rhs=W0t_sb[:], start=True, stop=True,
                        )
                    if jfull:
                        nc.vector.tensor_copy(
                            gst[:, j0:j0 + jfull, :], g0p[:, :jfull, :])
                    if jfull < jn:
                        rj = nn_ - (r0 + (j0 + jfull) * P)
                        nc.vector.tensor_copy(
                            gst[:rj, j0 + jfull, :], g0p[:rj, jfull, :])
                full = rn // P
                if full:
                    ov = g0_full[r0:r0 + full * P, :].rearrange(
                        "(j p) d -> p j d", p=P)
                    nc.sync.dma_start(ov, gst[:, :full, :])
                if rn % P:
                    nc.sync.dma_start(
                        g0_full[r0 + full * P:r0 + rn, :],
                        gst[:rn % P, full, :])

            # ---------- layers ----------
            def layer(gtab, b_sb, loc, full_t, is_last):
                pv = gtab[:, :].rearrange("(a b) e -> a (b e)", b=2)
                st = None
                st_t0 = 0
                st_n = 0

                def flush_stage():
                    nonlocal st, st_n
                    if st is None or st_n == 0:
                        return
                    r0 = st_t0 * P
                    rn = min(st_n * P, npc - r0)
                    full = rn // P
                    if full:
                        ov = loc[r0:r0 + full * P, :].rearrange(
                            "(j p) e -> p j e", p=P)
                        nc.sync.dma_start(ov, st[:, :full, :])
                    if rn % P:
                        nc.sync.dma_start(
                            loc[r0 + full * P:r0 + rn, :],
                            st[:rn % P, full, :])
                    st = None
                    st_n = 0

                def finish_tile(t, acc):
                    nonlocal st, st_t0, st_n
                    z = wpool.tile([P, d], f32, tag="z")
                    nc.vector.scalar_tensor_tensor(
                        out=z[:], in0=acc[:], scalar=invd_sb[:, t:t + 1],
                        in1=b_sb[:], op0=mybir.AluOpType.mult,
                        op1=mybir.AluOpType.add,
                    )
                    if st is None:
                        st = spool.tile([P, stb, d], bf16, tag="lst")
                        st_t0 = t
                        st_n = 0
                    if not is_last:
                        hf = wpool.tile([P, d], f32, tag="hf")
                        nc.scalar.activation(
                            hf[:], z[:], mybir.ActivationFunctionType.Tanh)
                        tp = ptp.tile([d, P], f32, tag="ptp")
                        nc.tensor.transpose(tp[:], hf[:], ident[:])
                        hT = wpool.tile([d, P], bf16, tag="hT")
                        nc.scalar.copy(hT[:], tp[:])
                        gm = pg.tile([P, d], f32, tag="pg")
                        nc.tensor.matmul(gm[:], lhsT=hT[:], rhs=W1t_sb[:],
                                         start=True, stop=True)
                        nc.scalar.copy(st[:, st_n, :], gm[:])
                    else:
                        hb = wpool.tile([P, d], bf16, tag="hb")
                        nc.scalar.activation(
                            hb[:], z[:], mybir.ActivationFunctionType.Tanh)
                        nc.vector.tensor_copy(st[:, st_n, :], hb[:])
                    st_n += 1
                    if st_n == stb or t == nt - 1:
                        flush_stage()

                acc_open = {}
                for w in range(CCp // gwc):
                    c0 = w * gwc
                    gt = gpool.tile([P, gwc, 2 * d], bf16, tag="gath")
                    nc.gpsimd.dma_gather(
                        out_ap=gt[:, :, :], in_ap=pv,
                        idxs_ap=sidx_sb[:, c0 * 8:(c0 + gwc) * 8],
                        num_idxs=gwc * P, num_idxs_reg=gwc * P,
                        elem_size=2 * d,
                    )
                    oh = ohpool.tile([P, gwc, P], bf16, tag="oh")
                    nc.vector.tensor_tensor(
                        out=oh[:, :, :],
                        in0=iota_b[:, None, :].broadcast_to([P, gwc, P]),
                        in1=dloc_sb[:, c0:c0 + gwc][:, :, None]
                            .broadcast_to([P, gwc, P]),
                        op=mybir.AluOpType.is_equal,
                    )
                    for lc in range(gwc):
                        col = c0 + lc
                        if col >= CC:
                            break
                        t = tile_of_col[col]
                        if t is None:
                            continue
                        first = col == col_base[t]
                        last = col == col_base[t] + tcols[t] - 1
                        if first:
                            acc_open[t] = pacc.tile([P, d], f32, tag="pacc",
                                                    name="acc")
                        ph = par_of_col[col]
                        nc.tensor.matmul(
                            acc_open[t][:],
                            lhsT=oh[:, lc, :],
                            rhs=gt[:, lc, ph * d:(ph + 1) * d],
                            start=first, stop=last,
                        )
                        if last:
                            finish_tile(t, acc_open.pop(t))
                flush_stage()
                nc.gpsimd.collective_compute(
                    "AllGather", mybir.AluOpType.bypass,
                    replica_groups=groups,
                    ins=[loc[:, :]], outs=[full_t[:, :]],
                )

            layer(g0_full, b0_sb, g1_loc, g1_full, is_last=False)
            layer(g1_full, b1_sb, h2_loc, h2_full, is_last=True)

            # ---------- scoring ----------
            hv = h2_full[:, :].rearrange("(a b) e -> a (b e)", b=2)
            sc_out = cpool.tile([P, nsc2], f32)
            ug = scpool.tile([P, nsc2p, 2 * d], bf16, tag="ug")
            ig = scpool.tile([P, nsc2p, 2 * d], bf16, tag="ig")
            for w in range(nsc2p // gwc):
                c0 = w * gwc
                nc.gpsimd.dma_gather(
                    out_ap=ug[:, c0:c0 + gwc, :], in_ap=hv,
                    idxs_ap=uidx_sb[:, c0 * 8:(c0 + gwc) * 8],
                    num_idxs=gwc * P, num_idxs_reg=gwc * P, elem_size=2 * d,
                )
                nc.gpsimd.dma_gather(
                    out_ap=ig[:, c0:c0 + gwc, :], in_ap=hv,
                    idxs_ap=iidx_sb[:, c0 * 8:(c0 + gwc) * 8],
                    num_idxs=gwc * P, num_idxs_reg=gwc * P, elem_size=2 * d,
                )
            for (s0, gc, pu, pi_) in sgroups:
                if gc == 0:
                    continue
                prod = scpool.tile([P, gc, d], bf16, tag="prod")
                nc.vector.tensor_tensor(
                    out=prod[:, :, :],
                    in0=ug[:, s0:s0 + gc, pu * d:(pu + 1) * d],
                    in1=ig[:, s0:s0 + gc, pi_ * d:(pi_ + 1) * d],
                    op=mybir.AluOpType.mult,
                )
                nc.vector.tensor_reduce(
                    out=sc_out[:, s0:s0 + gc], in_=prod[:, :, :],
                    axis=mybir.AxisListType.X, op=mybir.AluOpType.add,
                )
            nc.sync.dma_start(score_d[:, :], sc_out[:])

    nc.compile()
    return nc


class _SpmdRunner:
    """Mirrors bass2jax.run_bass_via_pjrt but caches the jitted executable so
    repeated kernel() calls skip re-tracing."""

    def __init__(self, nc, n_cores):
        import jax
        from jax.sharding import Mesh, PartitionSpec
        from jax.experimental.shard_map import shard_map
        import concourse.mybir as mybir
        from concourse import bass2jax
        from concourse.bass2jax import _bass_exec_p, install_neuronx_cc_hook

        install_neuronx_cc_hook()
        self.jax = jax
        self.n_cores = n_cores
        partition_name = (
            nc.partition_id_tensor.name if nc.partition_id_tensor else None
        )
        in_names, out_names, out_avals, zero_outs = [], [], [], []
        for alloc in nc.m.functions[0].allocations:
            if not isinstance(alloc, mybir.MemoryLocationSet):
                continue
            name = alloc.memorylocations[0].name
            if alloc.kind == "ExternalInput":
                if name != partition_name:
                    in_names.append(name)
            elif alloc.kind == "ExternalOutput":
                out_names.append(name)
                shape = tuple(alloc.tensor_shape)
                dtype = mybir.dt.np(alloc.dtype)
                out_avals.append(jax.core.ShapedArray(shape, dtype))
                zero_outs.append(np.zeros(shape, dtype))
        self.in_names, self.out_names = in_names, out_names
        self.out_avals, self.zero_outs = out_avals, zero_outs
        all_in_names = list(in_names) + list(out_names)
        if partition_name is not None:
            all_in_names.append(partition_name)

        def _body(*args):
            operands = list(args)
            if partition_name is not None:
                operands.append(bass2jax.partition_id_tensor())
            return tuple(_bass_exec_p.bind(
                *operands,
                out_avals=tuple(out_avals),
                in_names=tuple(all_in_names),
                out_names=tuple(out_names),
                lowering_input_output_aliases=(),
                sim_require_finite=True,
                sim_require_nnan=True,
                nc=nc,
            ))

        devices = jax.devices()[:n_cores]
        self.mesh = Mesh(np.asarray(devices), ("core",))
        n_io = len(in_names) + len(out_names)
        self.fn = jax.jit(
            shard_map(_body, mesh=self.mesh,
                      in_specs=(PartitionSpec("core"),) * n_io,
                      out_specs=(PartitionSpec("core"),) * len(out_names),
                      check_rep=False),
            keep_unused=True,
        )

    def run(self, in_maps):
        concat_in = [
            np.concatenate([np.asarray(in_maps[c][nm]) for c in range(self.n_cores)],
                           axis=0)
            for nm in self.in_names
        ]
        concat_zeros = [
            np.zeros((self.n_cores * z.shape[0], *z.shape[1:]), z.dtype)
            for z in self.zero_outs
        ]
        outs = self.fn(*concat_in, *concat_zeros)
        self.jax.block_until_ready(outs)
        res = [dict() for _ in range(self.n_cores)]
        for i, nm in enumerate(self.out_names):
            full = np.asarray(outs[i]).reshape(self.n_cores, *self.out_avals[i].shape)
            for c in range(self.n_cores):
                res[c][nm] = full[c]
        return res


_CACHE = {}


def _get_runner(cfg_key, cfg, meta):
    key = (cfg_key, meta["CC"], meta["tcols"], meta["nsc2"], meta["sgroups"])
    if key not in _CACHE:
        nc = build_nc(cfg, meta)
        _CACHE[key] = _SpmdRunner(nc, cfg["n_cores"])
    return _CACHE[key]


def assemble_score(results, cfg, meta):
    npc, nt, rows = derived(cfg)
    k = cfg["k"]
    parts = []
    for c, r in enumerate(results):
        sc = r["score"]                    # [P, nsc2]; position i = col*128 + p
        flat = np.ascontiguousarray(sc.T).ravel()
        perm = meta["perms"][c]
        out = np.zeros(rows * k, np.float32)
        m = perm >= 0
        out[perm[m]] = flat[m]
        parts.append(out.reshape(rows, k))
    return np.concatenate(parts, axis=0).astype(np.float32)


def run(inputs, cfg=None):
    """Returns (score [batch, k] float32, per-core results)."""
    cfg = dict(DEFAULT_CFG, **(cfg or {}))
    in_maps, meta = preprocess(inputs, cfg)
    cfg_key = tuple(sorted((kk, v) for kk, v in cfg.items()))
    runner = _get_runner(cfg_key, cfg, meta)
    results = runner.run(in_maps)
    return assemble_score(results, cfg, meta), results


def kernel(**inputs) -> np.ndarray:
    score, _ = run(inputs)
    return score
